# revision 7
# baseline (speedup 1.0000x reference)
"""Trainium2 Bass kernel for nn_Decoder: Linear + 4-layer GRU + tanh. v7.

Sharding: pure data parallel over batch, 32 samples per core on 8 cores.

v7 (vs v6): per-STEP PSUM banks with 4-deep rotation to kill the
head-of-line stall found in the v6 trace (at every 2-step group start
the bias matmul start=True waited on the sigmoid of the group two back;
~700us/core of PE idle, which also kept the PE HAM-throttled at 1.2GHz
for 2.07ms of the 2.40ms run).

Structure per step u (t = u - 8l per layer, SKEW=8 pipeline):
  pgA(u) [128, l4 x (r0 r1 z0 z1) x b32]   one bank
  pgB(u) [128, l4 x (nh0 nh1 nx0 nx1) x b32] one bank
Banks are allocated two steps ahead (at even u, banks for steps u+2,
u+3): one 16-row selector bias matmul per bank (start=True, clears
bank), then x-side matmuls for both steps sharing one LDWEIGHTS per
(l,q,j) with two N=32 matmuls (one per step bank). h-side matmuls for
step u run against banks allocated at u-2, so the PE queue always has
~2 steps of ready work and bias matmuls wait on chains 4 steps old.

Gate chain per step: sigmoid reads pgA directly (frees it first);
pgB is evacuated by one vector copy to bf16 SBUF (gB), freeing the
bank immediately and letting rhn/npre run in 2x bf16 mode.
Gate elementwise at layer-pair granularity:
sigmoid/tanh on Scalar, rhn/npre/nom/h' on Vector, omz/zh on GpSimd.
"""

import sys

sys.path.insert(0, "/opt/trn_rl_repo")

import numpy as np

import concourse.bass as bass
import concourse.tile as tile
from concourse import bacc, mybir
from concourse.bass_utils import run_bass_kernel_spmd

F32 = mybir.dt.float32
BF16 = mybir.dt.bfloat16

N_CORES = 8
B = 32
H = 256
HH = 128
Z = 128
L = 4
SKEW = 8
RING = 8
T_FULL = 512


def build_program(T=T_FULL, num_devices=N_CORES, phase0_lead=14, u_cap=None):
    U = T + (L - 1) * SKEW
    n_grp = T // 8
    n_win = max(1, T // 64)
    win_t = T // n_win
    feat = T * H

    nc = bacc.Bacc("TRN2", target_bir_lowering=False, debug=False,
                   num_devices=num_devices)

    d = {}
    d["zt"] = nc.dram_tensor("zt", [Z, B], BF16, kind="ExternalInput").ap()
    d["wlin"] = nc.dram_tensor("wlin", [Z, feat], BF16, kind="ExternalInput").ap()
    d["blin"] = nc.dram_tensor("blin", [n_grp, 16, HH], BF16, kind="ExternalInput").ap()
    d["bsel"] = nc.dram_tensor("bsel", [16, 512], BF16, kind="ExternalInput").ap()
    d["whh"] = nc.dram_tensor("whh", [HH, L * 12 * HH], BF16, kind="ExternalInput").ap()
    d["wih"] = nc.dram_tensor("wih", [HH, L * 12 * HH], BF16, kind="ExternalInput").ap()
    # per-step bank biases: rows (l,chunk) -> 16 rows
    d["bA"] = nc.dram_tensor("bA", [16, HH], BF16, kind="ExternalInput").ap()
    d["bB"] = nc.dram_tensor("bB", [16, HH], BF16, kind="ExternalInput").ap()
    d["selA"] = nc.dram_tensor("selA", [16, 512], BF16, kind="ExternalInput").ap()
    d["selB"] = nc.dram_tensor("selB", [16, 512], BF16, kind="ExternalInput").ap()
    d["out"] = nc.dram_tensor("out", [HH, T * 2 * B], F32, kind="ExternalOutput").ap()

    if u_cap is not None:
        U = u_cap
    with tile.TileContext(nc) as tc:
        _build(tc, nc, T, U, n_grp, n_win, win_t, phase0_lead, d)

    nc.compile()
    return nc


def _build(tc, nc, T, U, n_grp, n_win, win_t, lead, d):
    from contextlib import ExitStack
    ctx = ExitStack()

    res = ctx.enter_context(tc.tile_pool(name="res", bufs=1))
    wlinp = ctx.enter_context(tc.tile_pool(name="wlinp", bufs=3))
    pgAp = ctx.enter_context(tc.tile_pool(name="pgAp", bufs=4, space="PSUM"))
    pgBp = ctx.enter_context(tc.tile_pool(name="pgBp", bufs=4, space="PSUM"))
    gate = ctx.enter_context(tc.tile_pool(name="gate", bufs=4))
    outp = ctx.enter_context(tc.tile_pool(name="outp", bufs=2))

    SIG = mybir.ActivationFunctionType.Sigmoid
    TANH = mybir.ActivationFunctionType.Tanh
    MM = dict(skip_group_check=True)

    # ---- resident SBUF ----
    zt_sb = res.tile([Z, B], BF16, tag="zt")
    bsel_sb = res.tile([16, 512], BF16, tag="bsel")
    whh_sb = res.tile([HH, L * 12 * HH], BF16, tag="whh")
    wih_sb = res.tile([HH, L * 12 * HH], BF16, tag="wih")
    bA_sb = res.tile([16, HH], BF16, tag="bA")
    bB_sb = res.tile([16, HH], BF16, tag="bB")
    selA_sb = res.tile([16, 512], BF16, tag="selA")
    selB_sb = res.tile([16, 512], BF16, tag="selB")
    xw = [[res.tile([HH, win_t * B], BF16, tag=f"xw{q}_{w}", name=f"xw{q}_{w}")
           for w in range(n_win)] for q in range(2)]
    hR = res.tile([HH, L * 2 * RING * B], BF16, tag="hR")

    for name, t_sb in [("zt", zt_sb), ("bsel", bsel_sb), ("whh", whh_sb),
                       ("wih", wih_sb), ("bA", bA_sb), ("bB", bB_sb),
                       ("selA", selA_sb), ("selB", selB_sb)]:
        nc.sync.dma_start(t_sb[:], d[name][:])
    nc.vector.memset(hR[:], 0.0)

    d_out = d["out"]

    def wslice(wsb, l, q, j):
        c = ((l * 2 + q) * 6 + j) * HH
        return wsb[:, c:c + HH]

    def hslot(l, q, slot, n=1):
        c = ((l * 2 + q) * RING + slot) * B
        return hR[:, c:c + n * B]

    def lrange(w):
        la = max(0, -(-(w - (T - 1)) // SKEW))
        lb = min(L - 1, w // SKEW)
        return la, lb

    # ---- phase 0 ----
    ph0_tiles = {}

    def emit_ph0_dma(g):
        t0 = g * 8
        wl = wlinp.tile([Z, 8 * H], BF16, tag="wl", name="wl")
        nc.sync.dma_start(wl[:], d["wlin"][:, t0 * H:(t0 + 8) * H])
        bl = wlinp.tile([16, HH], BF16, tag="bl", name="bl")
        nc.sync.dma_start(bl[:], d["blin"][g])
        ph0_tiles[g] = (wl, bl)

    def emit_ph0_mm(g):
        t0 = g * 8
        wl, bl = ph0_tiles.pop(g)
        # px borrows from the pgA pool: its prior occupant's reader is the
        # sigmoid (first chain op), so the start=True bias matmul below
        # never waits long.
        px = pgAp.tile([HH, 512], F32, tag="pgA", name="px")
        nc.tensor.matmul(px[:], bl[:], bsel_sb[:], start=True, stop=False,
                         **MM)
        for q in range(2):
            for tp in range(8):
                nc.tensor.matmul(
                    px[:, q * 256 + tp * B: q * 256 + (tp + 1) * B],
                    wl[:, tp * H + q * HH: tp * H + (q + 1) * HH],
                    zt_sb[:],
                    start=False, stop=(q == 1 and tp == 7), **MM)
        w = t0 // win_t
        c0 = (t0 % win_t) * B
        nc.scalar.copy(xw[0][w][:, c0:c0 + 8 * B], px[:, 0:256])
        nc.vector.tensor_copy(xw[1][w][:, c0:c0 + 8 * B], px[:, 256:512])

    for g in range(min(lead, n_grp)):
        emit_ph0_dma(g)
        emit_ph0_mm(g)
    next_dma = min(lead, n_grp)
    next_mm = min(lead, n_grp)

    # ---- per-step bank groups, allocated 2 steps ahead ----
    banks = {}

    def emit_group_pair(w):
        """Allocate pgA/pgB banks for steps w, w+1 (w even); emit bias and
        x-side matmuls. One LDWEIGHTS per (l,q,j), two N=32 matmuls."""
        la, lb = lrange(w)
        # allocate each pair in REVERSED order: with 2 allocs per 2 steps
        # and bufs=4 this makes every bank's reuse lag exactly 4 steps
        # (in-order allocation gives the odd bank a 3-alloc lag, so its
        # start=True bias matmul waits on the previous step's chain and
        # head-of-line blocks the PE ~1.5us at every even u).
        pA1 = pgAp.tile([HH, 512], F32, tag="pgA", name=f"pgA{w + 1}")
        pA0 = pgAp.tile([HH, 512], F32, tag="pgA", name=f"pgA{w}")
        pB1 = pgBp.tile([HH, 512], F32, tag="pgB", name=f"pgB{w + 1}")
        pB0 = pgBp.tile([HH, 512], F32, tag="pgB", name=f"pgB{w}")
        pA = [pA0, pA1]
        pB = [pB0, pB1]
        for i in range(2):
            banks[w + i] = (pA[i], pB[i])
            nc.tensor.matmul(pA[i][:], bA_sb[:], selA_sb[:],
                             start=True, stop=False, **MM)
            nc.tensor.matmul(pB[i][:], bB_sb[:], selB_sb[:],
                             start=True, stop=False, **MM)
        for l in range(la, lb + 1):
            t0 = w - SKEW * l
            for q in range(2):
                if l == 0:
                    wv = t0 // win_t
                    c0 = (t0 % win_t) * B
                    if (t0 % win_t) <= win_t - 2:
                        srcs = [(xw[q][wv][:, c0 + i * B:c0 + (i + 1) * B], i)
                                for i in range(2)]
                    else:
                        srcs = [(xw[q][wv][:, c0:c0 + B], 0),
                                (xw[q][wv + 1][:, 0:B], 1)]
                else:
                    srcs = [(hslot(l - 1, q, (t0 + i) % RING), i)
                            for i in range(2)]
                for j in range(4):
                    wsl = wslice(wih_sb, l, q, j)
                    for srcap, i in srcs:
                        nc.tensor.matmul(
                            pA[i][:, l * HH + j * B: l * HH + (j + 1) * B],
                            wsl, srcap, start=False, stop=False, **MM)
                for k in range(2):
                    wsl = wslice(wih_sb, l, q, 4 + k)
                    for srcap, i in srcs:
                        nc.tensor.matmul(
                            pB[i][:, l * HH + (2 + k) * B:
                                  l * HH + (3 + k) * B],
                            wsl, srcap, start=False, stop=False, **MM)

    emit_group_pair(0)

    # ---- main loop ----
    for u in range(0, U):
        if u % 8 == 0 and next_dma < n_grp:
            emit_ph0_dma(next_dma)
            next_dma += 1
        if u % 8 == 4 and next_mm < n_grp:
            emit_ph0_mm(next_mm)
            next_mm += 1
        if u % 2 == 0 and u + 2 < U:
            emit_group_pair(u + 2)

        la, lb = lrange(u)
        slot = u % RING
        pslot = (u - 1) % RING
        pgA, pgB = banks.pop(u)

        # h-side matmuls (N=32), pair0 first
        for l in range(la, lb + 1):
            for q in range(2):
                src = hslot(l, q, pslot)
                for j in range(4):
                    nc.tensor.matmul(
                        pgA[:, l * HH + j * B: l * HH + (j + 1) * B],
                        wslice(whh_sb, l, q, j), src,
                        start=False,
                        stop=(l == lb and q == 1 and j == 3),
                        **MM)
                for k in range(2):
                    nc.tensor.matmul(
                        pgB[:, l * HH + k * B: l * HH + (k + 1) * B],
                        wslice(whh_sb, l, q, 4 + k), src,
                        start=False,
                        stop=(l == lb and q == 1 and k == 1),
                        **MM)

        # -- gates at layer-pair granularity --
        zr_t = gate.tile([HH, L * HH], BF16, tag="zr_t")
        n_t = gate.tile([HH, L * 64], BF16, tag="n_t")
        rhn = gate.tile([HH, L * 64], BF16, tag="rhn")
        npre = gate.tile([HH, L * 64], BF16, tag="npre")
        omz = gate.tile([HH, L * 64], BF16, tag="omz")
        zh = gate.tile([HH, L * 64], BF16, tag="zh")
        nom = gate.tile([HH, L * 64], BF16, tag="nom")

        hv = hR[:].rearrange("p (l q s b) -> p l q s b", l=L, q=2, s=RING)

        for p in range(la // 2, lb // 2 + 1):
            l0 = 2 * p
            nc.scalar.activation(zr_t[:, l0 * HH:(l0 + 2) * HH],
                                 pgA[:, l0 * HH:(l0 + 2) * HH],
                                 SIG)
            rv = zr_t[:, l0 * HH:(l0 + 2) * HH] \
                .rearrange("p (l c) -> p l c", l=2)[:, :, 0:64] \
                .rearrange("p l (k b) -> p l k b", k=2)
            pBh = pgB[:, l0 * HH:(l0 + 2) * HH] \
                .rearrange("p (lo g b) -> p lo g b", lo=2, g=4)
            nc.vector.tensor_mul(
                rhn[:, l0 * 64:(l0 + 2) * 64]
                .rearrange("p (l k b) -> p l k b", l=2, k=2),
                rv, pBh[:, :, 0:2])
            nc.vector.tensor_add(
                npre[:, l0 * 64:(l0 + 2) * 64]
                .rearrange("p (l k b) -> p l k b", l=2, k=2),
                rhn[:, l0 * 64:(l0 + 2) * 64]
                .rearrange("p (l k b) -> p l k b", l=2, k=2),
                pBh[:, :, 2:4])
            zv = zr_t[:].rearrange("p (l c) -> p l c", l=L)[:, l0:l0 + 2, 64:128] \
                .rearrange("p l (q b) -> p l q b", q=2)
            nc.gpsimd.tensor_scalar(
                out=omz[:, l0 * 64:(l0 + 2) * 64]
                .rearrange("p (l q b) -> p l q b", l=2, q=2),
                in0=zv, scalar1=-1.0, scalar2=1.0,
                op0=mybir.AluOpType.mult, op1=mybir.AluOpType.add)
            nc.gpsimd.tensor_mul(
                zh[:, l0 * 64:(l0 + 2) * 64]
                .rearrange("p (l q b) -> p l q b", l=2, q=2),
                zv, hv[:, l0:l0 + 2, :, pslot])
            nc.scalar.activation(n_t[:, l0 * 64:(l0 + 2) * 64],
                                 npre[:, l0 * 64:(l0 + 2) * 64], TANH)
            nc.vector.tensor_mul(nom[:, l0 * 64:(l0 + 2) * 64],
                                 omz[:, l0 * 64:(l0 + 2) * 64],
                                 n_t[:, l0 * 64:(l0 + 2) * 64])
            nc.vector.tensor_add(
                hv[:, l0:l0 + 2, :, slot],
                nom[:, l0 * 64:(l0 + 2) * 64]
                .rearrange("p (l q b) -> p l q b", l=2, q=2),
                zh[:, l0 * 64:(l0 + 2) * 64]
                .rearrange("p (l q b) -> p l q b", l=2, q=2))

        # boundary fix: the pair-granular gate ops above write garbage h'
        # for layers that are not yet active; re-zero the slot layer lam
        # will read as h_{-1} on its first step (u = 8*lam).
        if u % SKEW == SKEW - 1:
            lam = (u + 1) // SKEW
            if 1 <= lam <= L - 1 and lam > lb:
                nc.vector.memset(hv[:, lam, :, (SKEW * lam - 1) % RING], 0.0)

        # -- output: layer 3, tanh batched over 4 steps --
        if lb == L - 1:
            t3 = u - (L - 1) * SKEW
            if t3 % 32 == 0:
                ob_cur = outp.tile([HH, 32 * 2 * B], F32, tag="ob")
            ob = ob_cur
            if t3 % 4 == 3 or t3 == T - 1:
                tb0 = (t3 // 4) * 4
                nn = t3 - tb0 + 1
                s0 = tb0 % RING
                nc.scalar.activation(
                    ob[:, (tb0 % 32) * 64:(tb0 % 32 + nn) * 64]
                    .rearrange("p (t q b) -> p t q b", t=nn, q=2),
                    hv[:, L - 1, :, s0:s0 + nn]
                    .rearrange("p q t b -> p t q b"),
                    TANH)
            if t3 % 32 == 31 or t3 == T - 1:
                w0 = (t3 // 32) * 32
                nw = t3 - w0 + 1
                nc.sync.dma_start(d_out[:, w0 * 64:(w0 + nw) * 64],
                                  ob[:, 0:nw * 64])

    ctx.close()


# ------------------------------------------------------------------
# host-side packing
# ------------------------------------------------------------------

def _pack_inputs(z, W_lin, b_lin, W_ih, W_hh, b_ih, b_hh, T=T_FULL):
    import ml_dtypes
    bf = ml_dtypes.bfloat16
    n_grp = T // 8

    W_linT = np.ascontiguousarray(np.asarray(W_lin).T).astype(bf)
    blin = np.ascontiguousarray(
        np.asarray(b_lin).reshape(n_grp, 8, 2, HH).transpose(0, 2, 1, 3)
        .reshape(n_grp, 16, HH)).astype(bf)
    bsel = np.zeros((16, 512), np.float32)
    for k in range(16):
        q, tp = k // 8, k % 8
        bsel[k, q * 256 + tp * B: q * 256 + (tp + 1) * B] = 1.0

    def packw(Wm):
        out = np.empty((HH, L * 12 * HH), np.float32)
        Wm = np.asarray(Wm)
        for l in range(L):
            for q in range(2):
                for j in range(6):
                    c = ((l * 2 + q) * 6 + j) * HH
                    out[:, c:c + HH] = Wm[l, j * HH:(j + 1) * HH,
                                          q * HH:(q + 1) * HH].T
        return out.astype(bf)

    whh = packw(W_hh)
    wih = packw(W_ih)

    b_ih = np.asarray(b_ih)
    b_hh = np.asarray(b_hh)
    # per-step banks [l4][chunk4][b32]; bias rows (l*4+chunk)
    bA = np.zeros((16, HH), np.float32)
    bB = np.zeros((16, HH), np.float32)
    selA = np.zeros((16, 512), np.float32)
    selB = np.zeros((16, 512), np.float32)
    for l in range(L):
        for j in range(4):
            bA[l * 4 + j] = (b_ih[l, j * HH:(j + 1) * HH]
                             + b_hh[l, j * HH:(j + 1) * HH])
            selA[l * 4 + j, l * HH + j * B: l * HH + (j + 1) * B] = 1.0
        for k in range(2):
            bB[l * 4 + k] = b_hh[l, (4 + k) * HH:(5 + k) * HH]
            bB[l * 4 + 2 + k] = b_ih[l, (4 + k) * HH:(5 + k) * HH]
            selB[l * 4 + k, l * HH + k * B: l * HH + (k + 1) * B] = 1.0
            selB[l * 4 + 2 + k,
                 l * HH + (2 + k) * B: l * HH + (3 + k) * B] = 1.0

    shared = dict(wlin=W_linT, blin=blin, bsel=bsel.astype(bf),
                  whh=whh, wih=wih,
                  bA=bA.astype(bf), bB=bB.astype(bf),
                  selA=selA.astype(bf), selB=selB.astype(bf))
    in_maps = []
    z = np.asarray(z)
    for c in range(N_CORES):
        zt = np.ascontiguousarray(z[c * B:(c + 1) * B, :].T).astype(bf)
        in_maps.append(dict(shared, zt=zt))
    return in_maps


def _unpack_out(raw, T=T_FULL):
    a = np.asarray(raw, np.float32).reshape(HH, T, 2, B)
    return np.transpose(a, (3, 1, 2, 0)).reshape(B, T * H)


_CACHED = {}


def kernel(z, W_lin, b_lin, W_ih, W_hh, b_ih, b_hh):
    import time as _time
    if "nc" not in _CACHED:
        t0 = _time.time()
        print("building program ...", flush=True)
        _CACHED["nc"] = build_program()
        print(f"build done in {_time.time() - t0:.0f}s", flush=True)
    nc = _CACHED["nc"]
    print("packing inputs ...", flush=True)
    in_maps = _pack_inputs(z, W_lin, b_lin, W_ih, W_hh, b_ih, b_hh)
    print("launching (compile on first call) ...", flush=True)
    res = run_bass_kernel_spmd(nc, in_maps, list(range(N_CORES)))
    print("run complete", flush=True)
    outs = [_unpack_out(res.results[c]["out"]) for c in range(N_CORES)]
    full = np.concatenate(outs, axis=0)
    return full.reshape(-1, H, T_FULL).astype(np.float32)
